# revision 23
# baseline (speedup 1.0000x reference)
"""Trainium2 Bass kernel for nn_Discriminator_80195629351349.

Pairwise-column MLP discriminator over k-space columns.

Math (matching the jax reference):
  F[b, w, ch] = |kspace[b, c, h, w]|  (ch = c*H + h)
  Pq = Fq @ W1[:, :CH].T ;  Pa = Fa @ W1[:, CH:].T          [B, W, 18]
  out[b, wi, wc] = sigmoid(W4 @ r3 + b4),  r3 = relu-chain of
                   relu(Pq[wi] + Pa[wc] + b1) through W2, W3
  heat[b, wi] = sum_wc out[b, wi, wc] * cmask[b, wc] / denom[b]
  result[b, h, w] = heat[b, w] if acquiring_mask[b, w] > 0 else 0

Only columns wi with acquiring_mask>0 (16 of 384) contribute to the
output, and the wc sum runs only over [left, right) (191 of 384
columns), so the kernel computes exactly that slice.

Sharding: 8 cores = (batch b in 0..3) x (wc half s in 0..1). Each core
gets its slice of acquired/acquiring k-space columns, pre-packed on
the host into bf16 in the exact SBUF layout (so every DMA is 128 fat
contiguous packets), computes features + all pair MLP evaluations
on-device, and returns partial heat sums [4, NL]. Host combines and
divides by denom.

On-device layout: the 18-channel MLP is packed 4x block-diagonal
across the 128 partitions (quadrant j = partitions 32j..32j+17), so
layers 2-4 are single matmuls with N = NL*NWC <= 512 free columns.
The q (acquiring) columns ride in the same feature stream as the a
(acquired) columns: per k-tile the rhs is [128, NWC+NS] and the W1
lhsT is [W1q_k | zeros | W1a_k] (50 wide) so one matmul chain yields
Pq (rows 0:18, cols NWC:) and Pa (rows 32:50, cols :NWC) in one PSUM.

The pair pre-activation H[32j+i, lw*NWC+c] = Pa[i,c] + Pq[i,j*NL+lw]
(+b1, and -30000 on pad columns) is built entirely on the PE: Pq is
transposed via the PE transpose path, scattered into a per-slot lhsT
L (tiny same-partition copies), and one matmul against a constant
selector adds the right Pq value to every pair column; four more
column-slice matmuls against REP replicate Pa. No SBUF->SBUF DMAs.

Padding: pad columns get -30000 via the selector's extra all-ones row,
forcing h1=0 there, so their sigmoid output is exactly sigmoid(b4);
the host subtracts that known constant.
"""

import math
import os

import numpy as np
import ml_dtypes

BF16 = ml_dtypes.bfloat16

B, C, H, W = 4, 8, 384, 384
CH = C * H            # 3072 features per column
P = 128               # SBUF partitions
KT = CH // P          # 24 contraction tiles
CHANS = 18            # MLP width
NCORES = 8
CHUNK_KS = (4, 5, 6, 4, 3, 2)   # k-tiles per stream chunk: small first
#                                   chunk (early compute start) and small
#                                   last chunk (short trailing chain)
NCHUNK = len(CHUNK_KS)
ACT_IM = (2, 3)       # chunks whose im^2 runs on ACT (Square) instead of DVE

_prog_cache: dict = {}
LAST_RESULTS = None   # BassKernelResults of the most recent run (for test.py)


def _cst_layout(NWC, NS, NF):
    """Column offsets of the constant block [128, CW] (bf16)."""
    o = {}
    off = 0
    o["W1Q"] = off; off += KT * CHANS         # per-k W1q_k lhsT
    o["W1A"] = off; off += KT * CHANS         # per-k W1a_k lhsT
    o["W2BD"] = off; off += P                 # block-diag W2.T
    o["W3BD"] = off; off += P
    o["W4BD"] = off; off += 4
    o["REP"] = off; off += P                  # rows 0:18, eye at 32j offsets
    o["PADP"] = off; off += NWC               # rows 0:18: -30000 at pad cols
    o["B1"] = off; off += 1
    o["B2"] = off; off += 1
    o["B3"] = off; off += 1
    o["B4"] = off; off += 1
    o["CW"] = off
    return o


def _build_program(NWC: int, NL: int):
    """Build the SPMD Bass/Tile program for one core.

    NWC: number of wc (acquired) columns this core handles.
    NL:  wi slots per partition-quadrant (total wi slots = 4*NL).
    """
    import concourse.bass as bass
    import concourse.tile as tile
    from concourse import bacc, mybir

    f32 = mybir.dt.float32
    bf16 = mybir.dt.bfloat16
    NS = 4 * NL          # wi slots
    NCOL = NWC + NS      # feature columns per k-tile (a block then q block)
    NF = NL * NWC        # free columns of the pair block
    o = _cst_layout(NWC, NS, NF)
    CW = o["CW"]
    KOFF = [sum(CHUNK_KS[:c]) for c in range(NCHUNK + 1)]
    assert NF <= 512

    nc = bacc.Bacc("TRN2", debug=False)

    # ---- DRAM I/O (per-core; host packs exactly these layouts) ----
    # xks partition row: [chunk][re/im][k-in-chunk][col] -> contiguous 1D
    xks = nc.dram_tensor("xks", [P, 2 * KT * NCOL], bf16, kind="ExternalInput")
    cst = nc.dram_tensor("cst", [P, CW], bf16, kind="ExternalInput")
    hp = nc.dram_tensor("hp", [4, NL], f32, kind="ExternalOutput")

    AF = mybir.ActivationFunctionType
    ALU = mybir.AluOpType

    halves = (NL % 2 == 0 and NL >= 2)
    with tile.TileContext(nc) as tc:
        with (
            tc.tile_pool(name="consts", bufs=1) as consts,
            tc.tile_pool(name="xdata", bufs=NCHUNK) as xdata,
            tc.tile_pool(name="sq", bufs=2) as sqp,
            tc.tile_pool(name="feat", bufs=1) as featp,
            tc.tile_pool(name="mlp", bufs=1) as mlp,
            tc.tile_pool(name="ps1", bufs=1, space="PSUM") as ps1,
            tc.tile_pool(name="psa", bufs=1, space="PSUM") as psap,
            tc.tile_pool(name="psT", bufs=1, space="PSUM") as psT,
            tc.tile_pool(name="psH", bufs=1, space="PSUM") as psH,
            tc.tile_pool(name="ps2", bufs=1, space="PSUM") as ps2p,
            tc.tile_pool(name="ps3", bufs=1, space="PSUM") as ps3p,
            tc.tile_pool(name="ps4", bufs=1, space="PSUM") as ps4p,
        ):
            # ---- ACT sqrt-table prefetch before any data lands ----
            dum = mlp.tile([1, 1], f32, tag="dum")
            nc.vector.memset(dum, 1.0)
            dsq = mlp.tile([1, 1], f32, tag="dsq")
            nc.scalar.sqrt(dsq, dum)
            # L: per-slot lhsT for the Pq scatter matmul; row 32j+lw
            # gets pqT quadrant block j (32-aligned partition bases)
            L = mlp.tile([P, P], bf16, tag="L")
            nc.vector.memset(L, 0.0)
            # Sel[32j+lw, lw*NWC+c] = 1 selector, built on device while
            # the DMAs stream: iota+compare for one 32-row quadrant, then
            # aligned partition copies
            Sel = mlp.tile([P, NF], bf16, tag="Sel")
            sit = mlp.tile([32, NF], bf16, tag="sit")
            nc.gpsimd.iota(sit.rearrange("p (l c) -> p l c", l=NL),
                           pattern=[[1, NL], [0, NWC]], base=0,
                           channel_multiplier=-1,
                           allow_small_or_imprecise_dtypes=True)
            nc.vector.tensor_scalar(out=Sel[0:32, :], in0=sit, scalar1=0.0,
                                    scalar2=None, op0=ALU.is_equal)
            for j in range(1, 4):
                nc.vector.tensor_copy(Sel[32 * j:32 * (j + 1), :], Sel[0:32, :])

            # ---- input DMAs: all chunks on the sync ring in stream
            # order (chunk0 must land first; sharing rings only delays
            # it), constants on the scalar ring ----
            xc = []
            for ci in range(NCHUNK):
                cw = CHUNK_KS[ci] * NCOL
                t = xdata.tile([P, 2, cw], bf16, tag=f"x{ci}", name=f"x{ci}")
                a = 2 * KOFF[ci] * NCOL
                nc.sync.dma_start(
                    out=t, in_=xks[:, a:a + 2 * cw].rearrange(
                        "p (r n) -> p r n", r=2))
                xc.append(t)
            cst_s = consts.tile([P, CW], bf16, tag="cst")
            nc.scalar.dma_start(out=cst_s, in_=cst[:])

            # ---- features: |z| = sqrt(re^2 + im^2), then W1 matmuls.
            # DVE does all re^2 + most im^2 + all adds; ACT (Square shares
            # every table with Sqrt, so no table swaps) takes the middle
            # chunks' im^2 so the last chunk's chain stays on DVE (ACT
            # would otherwise trail). Separate q/a psums so Pq's pipeline
            # can start before the last a-matmuls retire. ----
            feat = featp.tile([P, KT * NCOL], bf16, tag="feat")
            psq = ps1.tile([CHANS, NS], f32, tag="psq")
            psa = psap.tile([CHANS, NWC], f32, tag="psa")
            sq_re = [sqp.tile([P, CHUNK_KS[ci] * NCOL], bf16, tag=f"sqre{ci}",
                              name=f"sqre{ci}") for ci in range(NCHUNK)]
            sq_im = [sqp.tile([P, CHUNK_KS[ci] * NCOL], bf16, tag=f"sqim{ci}",
                              name=f"sqim{ci}") for ci in range(NCHUNK)]
            m2 = [sqp.tile([P, CHUNK_KS[ci] * NCOL], bf16, tag=f"m2{ci}",
                           name=f"m2{ci}") for ci in range(NCHUNK)]

            def mm_a(k):
                nc.tensor.matmul(
                    out=psa,
                    lhsT=cst_s[:, o["W1A"] + k * CHANS:
                               o["W1A"] + (k + 1) * CHANS],
                    rhs=feat[:, k * NCOL:k * NCOL + NWC],
                    start=(k == 0), stop=(k == KT - 1))

            def mm_q(k):
                nc.tensor.matmul(
                    out=psq,
                    lhsT=cst_s[:, o["W1Q"] + k * CHANS:
                               o["W1Q"] + (k + 1) * CHANS],
                    rhs=feat[:, k * NCOL + NWC:(k + 1) * NCOL],
                    start=(k == 0), stop=(k == KT - 1))

            def emit_mms(ci):
                ks = range(KOFF[ci], KOFF[ci + 1])
                # last chunk: q matmuls first so the Pq tail starts early
                if ci == NCHUNK - 1:
                    for k in ks:
                        mm_q(k)
                    for k in ks:
                        mm_a(k)
                else:
                    for k in ks:
                        mm_a(k)
                    for k in ks:
                        mm_q(k)

            def dve_sq(ci, im):
                nc.vector.tensor_mul(sq_re[ci], xc[ci][:, 0], xc[ci][:, 0])
                if im:
                    nc.vector.tensor_mul(sq_im[ci], xc[ci][:, 1], xc[ci][:, 1])

            def dve_add(ci):
                nc.vector.tensor_add(m2[ci], sq_re[ci], sq_im[ci])

            def act_sqrt(ci):
                nc.scalar.sqrt(
                    feat[:, KOFF[ci] * NCOL:KOFF[ci + 1] * NCOL], m2[ci])

            actim = [ci for ci in ACT_IM if ci < NCHUNK]
            for ci in range(NCHUNK):
                on_dve = ci not in actim
                dve_sq(ci, on_dve)
                if on_dve:
                    dve_add(ci)
                    act_sqrt(ci)
                    # slot an ACT square between sqrts while data streams
                    if actim:
                        nc.scalar.square(sq_im[actim[0]], xc[actim[0]][:, 1])
                        actim.pop(0)
                else:
                    dve_add(ci)
                    act_sqrt(ci)
                emit_mms(ci)

            # sigmoid ACT-table prefetch: chained on the last sqrt's output
            # so it runs after all sqrts (exactly one table swap)
            dsg = mlp.tile([1, 1], f32, tag="dsg")
            nc.scalar.activation(out=dsg, in_=feat[0:1, KT * NCOL - 1:KT * NCOL],
                                 func=AF.Sigmoid)

            # biases as f32 (scalar operands must be float32); on gpsimd
            # so the wait doesn't block the DVE queue
            bias_f = mlp.tile([P, 4], f32, tag="biasf")
            nc.gpsimd.tensor_copy(bias_f, cst_s[:, o["B1"]:o["B1"] + 4])
            b1_s = bias_f[0:CHANS, 0:1]
            b2_s = bias_f[:, 1:2]
            b3_s = bias_f[:, 2:3]
            b4_s = bias_f[0:4, 3:4]

            # ---- extract Pq (+b1) and Pa from their psums ----
            pq_s = mlp.tile([CHANS, NS], bf16, tag="pq")
            nc.vector.tensor_scalar(out=pq_s, in0=psq,
                                    scalar1=b1_s, scalar2=None, op0=ALU.add)

            # ---- Pq quadrant scatter without DMA: 4 per-quadrant PE
            # transposes into one PSUM bank (base 0), then 32-aligned
            # copies into L ----
            # Pa extraction first (ready as soon as the a-psum stops);
            # the add also injects -30000 into pad columns
            pa_s = mlp.tile([CHANS, NWC], bf16, tag="pa")
            nc.vector.tensor_add(pa_s, psa,
                                 cst_s[0:CHANS, o["PADP"]:o["PADP"] + NWC])
            pqT = psT.tile([NL, 4 * CHANS], bf16, tag="pqT")
            eye = cst_s[0:CHANS, o["REP"]:o["REP"] + CHANS]
            for j in range(4):
                nc.tensor.matmul(
                    out=pqT[:, j * CHANS:(j + 1) * CHANS],
                    lhsT=pq_s[:, j * NL:(j + 1) * NL], rhs=eye,
                    is_transpose=True, start=(j == 0), stop=True,
                    skip_group_check=True)
            for j in range(4):
                nc.vector.tensor_copy(
                    L[32 * j:32 * j + NL, 32 * j:32 * j + CHANS],
                    pqT[:, j * CHANS:(j + 1) * CHANS])

            # ---- H = Pa (replicated, includes pad penalty; pa is ready
            # before L) + Pq (scattered) ----
            Hps = psH.tile([P, NF], f32, tag="H")
            for lw in range(NL):
                nc.tensor.matmul(out=Hps[:, lw * NWC:(lw + 1) * NWC],
                                 lhsT=cst_s[0:CHANS, o["REP"]:o["REP"] + P],
                                 rhs=pa_s, start=(lw == 0), stop=False,
                                 skip_group_check=True)
            nc.tensor.matmul(out=Hps, lhsT=L, rhs=Sel,
                             start=False, stop=True, skip_group_check=True)

            # ---- pair MLP (two column-half pipelines when NL is even) ----
            h1 = mlp.tile([P, NF], bf16, tag="h1")
            h2 = mlp.tile([P, NF], bf16, tag="h2")
            h3 = mlp.tile([P, NF], bf16, tag="h3")
            sig = mlp.tile([4, NF], bf16, tag="sig")
            psum2 = ps2p.tile([P, NF], f32, tag="ps2")
            psum3 = ps3p.tile([P, NF], f32, tag="ps3")
            psum4 = ps4p.tile([4, NF], f32, tag="ps4")
            hp_s = mlp.tile([4, NL], f32, tag="hps")
            HNF = NF // 2 if halves else NF
            HNL = NL // 2 if halves else NL
            for hf in range(2 if halves else 1):
                sl = slice(hf * HNF, hf * HNF + HNF)
                nc.vector.tensor_scalar(out=h1[:, sl], in0=Hps[:, sl],
                                        scalar1=0.0, scalar2=None, op0=ALU.max)
                nc.tensor.matmul(out=psum2[:, sl],
                                 lhsT=cst_s[:, o["W2BD"]:o["W2BD"] + P],
                                 rhs=h1[:, sl], start=(hf == 0), stop=True,
                                 skip_group_check=True)
                # h2 relu on ACT (Relu lives in every table; ACT is idle
                # here and this decouples the two half-pipelines)
                nc.scalar.activation(out=h2[:, sl], in_=psum2[:, sl],
                                     func=AF.Relu, bias=b2_s, scale=1.0)
                nc.tensor.matmul(out=psum3[:, sl],
                                 lhsT=cst_s[:, o["W3BD"]:o["W3BD"] + P],
                                 rhs=h2[:, sl], start=(hf == 0), stop=True,
                                 skip_group_check=True)
                nc.vector.tensor_scalar(out=h3[:, sl], in0=psum3[:, sl],
                                        scalar1=b3_s, scalar2=0.0,
                                        op0=ALU.add, op1=ALU.max)
                nc.tensor.matmul(out=psum4[:, sl],
                                 lhsT=cst_s[:, o["W4BD"]:o["W4BD"] + 4],
                                 rhs=h3[:, sl], start=(hf == 0), stop=True,
                                 skip_group_check=True)
                nc.scalar.activation(out=sig[:, sl], in_=psum4[:, sl],
                                     func=AF.Sigmoid, bias=b4_s, scale=1.0)
                # heat[j, lw] = sum_c sig[j, lw*NWC + c]
                nc.vector.reduce_sum(
                    hp_s[:, hf * HNL:hf * HNL + HNL],
                    sig[:, sl].rearrange("p (l c) -> p l c", l=HNL),
                    axis=mybir.AxisListType.X)
            nc.sync.dma_start(out=hp[:], in_=hp_s)

    nc.finalize()
    return nc


def _run_sim(nc, in_maps):
    """CoreSim (CPU instruction simulator) path for local dev testing."""
    from concourse.bass_interp import MultiCoreSim
    from concourse.bass_utils import BassKernelResults

    sim = MultiCoreSim(nc, num_cores=len(in_maps))
    for core_id, core in sim.cores.items():
        for name, arr in in_maps[core_id].items():
            core.tensor(name)[:] = arr
    sim.simulate()
    results = [
        {"hp": np.array(sim.cores[i].tensor("hp"))} for i in range(len(in_maps))
    ]
    return BassKernelResults(results=results, instructions_and_trace=None,
                             profile_json=None, exec_time_ns=None)


def _mask_geometry(acquired_mask, acquiring_mask):
    """Replicates the reference's left/right/cmask/denom logic exactly."""
    am = np.asarray(acquired_mask, np.float32)
    qm = np.asarray(acquiring_mask, np.float32)
    mid = W // 2
    right = mid + np.argmax(am[:, mid:] < 1.0, axis=1)
    left = np.argmax(am[:, :mid][:, ::-1] < 1.0, axis=1) + 1
    cols = np.arange(W)
    cmask = (cols[None, :] >= left[:, None]) & (cols[None, :] < right[:, None])
    denom = (right - left).astype(np.float32)
    active = [np.nonzero(qm[b] > 0)[0] for b in range(B)]
    return left.astype(int), right.astype(int), cmask, denom, active


def kernel(acquired_kspace, acquiring_kspace, acquired_mask, acquiring_mask,
           W1, b1, W2, b2, W3, b3, W4, b4):
    global LAST_RESULTS
    from concourse.bass_utils import run_bass_kernel_spmd

    acquired_kspace = np.asarray(acquired_kspace, np.float32)
    acquiring_kspace = np.asarray(acquiring_kspace, np.float32)
    W1 = np.asarray(W1, np.float32)
    b1 = np.asarray(b1, np.float32)
    W2 = np.asarray(W2, np.float32)
    b2 = np.asarray(b2, np.float32)
    W3 = np.asarray(W3, np.float32)
    b3 = np.asarray(b3, np.float32)
    W4 = np.asarray(W4, np.float32)
    b4 = np.asarray(b4, np.float32)

    left, right, cmask, denom, active = _mask_geometry(acquired_mask, acquiring_mask)

    nmax = max(len(a) for a in active)
    out = np.zeros((B, H, W), np.float32)
    if nmax == 0:
        return out

    span = max(int((right - left).max()), 1)
    NL = max(1, math.ceil(nmax / 4))          # wi slots per quadrant
    NWC = max(1, math.ceil(span / 2))         # wc columns per core
    NS = 4 * NL
    NF = NL * NWC
    NCOL = NWC + NS
    assert NF <= 512, (NL, NWC)

    o = _cst_layout(NWC, NS, NF)

    # ---- shared constant block [128, CW] bf16 ----
    W1q, W1a = W1[:, :CH], W1[:, CH:]
    cstv = np.zeros((P, o["CW"]), np.float32)
    # separate per-k lhsT blocks: [p, k*18 + j] = W1x[j, k*128+p]
    w1q_t = W1q.T.reshape(KT, P, CHANS)       # [k, p, j]
    w1a_t = W1a.T.reshape(KT, P, CHANS)
    cstv[:, o["W1Q"]:o["W1Q"] + KT * CHANS] = (
        w1q_t.transpose(1, 0, 2).reshape(P, KT * CHANS))
    cstv[:, o["W1A"]:o["W1A"] + KT * CHANS] = (
        w1a_t.transpose(1, 0, 2).reshape(P, KT * CHANS))
    for j in range(4):
        sl = slice(32 * j, 32 * j + CHANS)
        cstv[sl, o["W2BD"] + 32 * j:o["W2BD"] + 32 * j + CHANS] = W2.T
        cstv[sl, o["W3BD"] + 32 * j:o["W3BD"] + 32 * j + CHANS] = W3.T
        cstv[sl, o["W4BD"] + j] = W4[0]
        cstv[sl, o["B2"]] = b2
        cstv[sl, o["B3"]] = b3
        cstv[:CHANS, o["REP"] + 32 * j:o["REP"] + 32 * j + CHANS] = np.eye(
            CHANS, dtype=np.float32)
    cstv[:CHANS, o["B1"]] = b1
    cstv[:4, o["B4"]] = float(b4[0])

    # ---- per-core packed inputs ----
    in_maps = []
    meta = []
    npad = np.zeros((B, 2), np.int64)
    for b in range(B):
        aw = active[b]
        awp = np.zeros(NS, np.int64)
        if len(aw):
            awp[:len(aw)] = aw
            awp[len(aw):] = aw[0]
        # acquiring features for active wi columns: [KT, P, NS, 2]
        Q = acquiring_kspace[b].reshape(CH, W, 2)[:, awp, :].reshape(
            KT, P, NS, 2)
        for s in range(2):
            w0 = int(left[b]) + s * NWC
            w1e = max(min(w0 + NWC, int(right[b])), w0)
            nv = w1e - w0
            npad[b, s] = NWC - nv
            A = np.zeros((KT, P, NWC, 2), np.float32)
            if nv > 0:
                A[:, :, :nv, :] = acquired_kspace[b].reshape(CH, W, 2)[
                    :, w0:w1e, :].reshape(KT, P, nv, 2)
            F = np.concatenate([A, Q], axis=2)      # [KT, P, NCOL, 2]
            # -> per chunk [P, r, k-in-chunk, col], chunk-major flattened
            parts = []
            koff = 0
            for kc in CHUNK_KS:
                Xc = F[koff:koff + kc].transpose(1, 3, 0, 2)  # [P, 2, kc, NCOL]
                parts.append(Xc.reshape(P, 2 * kc * NCOL))
                koff += kc
            xarr = np.ascontiguousarray(
                np.concatenate(parts, axis=1)).astype(BF16)
            cstc = cstv.copy()
            if nv < NWC:
                cstc[0:CHANS, o["PADP"] + nv:o["PADP"] + NWC] = -30000.0
            in_maps.append(dict(xks=xarr, cst=cstc.astype(BF16)))
            meta.append((b, s))

    key = (NWC, NL)
    if key not in _prog_cache:
        _prog_cache[key] = _build_program(NWC, NL)
    nc = _prog_cache[key]

    trace = bool(int(os.environ.get("CABSK_TRACE", "0")))
    tmpdir = os.environ.get("CABSK_TMPDIR") or None
    if tmpdir:
        import tempfile
        tmpdir = tempfile.mkdtemp(dir=tmpdir)
    if os.environ.get("CABSK_SIM", "0") == "1":
        res = _run_sim(nc, in_maps)
    else:
        res = run_bass_kernel_spmd(nc, in_maps, core_ids=list(range(NCORES)),
                                   trace=trace, tmpdir=tmpdir)
    LAST_RESULTS = res

    sig_b4 = 1.0 / (1.0 + math.exp(-float(b4[0])))  # pad columns' output
    heat = np.zeros((B, W), np.float32)
    for ci, (b, s) in enumerate(meta):
        hpv = np.asarray(res.results[ci]["hp"], np.float32)   # [4, NL]
        aw = active[b]
        corr = float(npad[b, s]) * sig_b4
        for t in range(len(aw)):
            heat[b, aw[t]] += hpv[t // NL, t % NL] - corr
    heat /= np.maximum(denom, 1.0)[:, None]
    out[:] = heat[:, None, :]
    return out


# revision 24
# speedup vs baseline: 1.0195x; 1.0195x over previous
"""Trainium2 Bass kernel for nn_Discriminator_80195629351349.

Pairwise-column MLP discriminator over k-space columns.

Math (matching the jax reference):
  F[b, w, ch] = |kspace[b, c, h, w]|  (ch = c*H + h)
  Pq = Fq @ W1[:, :CH].T ;  Pa = Fa @ W1[:, CH:].T          [B, W, 18]
  out[b, wi, wc] = sigmoid(W4 @ r3 + b4),  r3 = relu-chain of
                   relu(Pq[wi] + Pa[wc] + b1) through W2, W3
  heat[b, wi] = sum_wc out[b, wi, wc] * cmask[b, wc] / denom[b]
  result[b, h, w] = heat[b, w] if acquiring_mask[b, w] > 0 else 0

Only columns wi with acquiring_mask>0 (16 of 384) contribute to the
output, and the wc sum runs only over [left, right) (191 of 384
columns), so the kernel computes exactly that slice.

Sharding: 8 cores = (batch b in 0..3) x (wc half s in 0..1). Each core
gets its slice of acquired/acquiring k-space columns, pre-packed on
the host into bf16 in the exact SBUF layout (so every DMA is 128 fat
contiguous packets), computes features + all pair MLP evaluations
on-device, and returns partial heat sums [4, NL]. Host combines and
divides by denom.

On-device layout: the 18-channel MLP is packed 4x block-diagonal
across the 128 partitions (quadrant j = partitions 32j..32j+17), so
layers 2-4 are single matmuls with N = NL*NWC <= 512 free columns.
The q (acquiring) columns ride in the same feature stream as the a
(acquired) columns: per k-tile the rhs is [128, NWC+NS] and the W1
lhsT is [W1q_k | zeros | W1a_k] (50 wide) so one matmul chain yields
Pq (rows 0:18, cols NWC:) and Pa (rows 32:50, cols :NWC) in one PSUM.

The pair pre-activation H[32j+i, lw*NWC+c] = Pa[i,c] + Pq[i,j*NL+lw]
(+b1, and -30000 on pad columns) is built entirely on the PE: Pq is
transposed via the PE transpose path, scattered into a per-slot lhsT
L (tiny same-partition copies), and one matmul against a constant
selector adds the right Pq value to every pair column; four more
column-slice matmuls against REP replicate Pa. No SBUF->SBUF DMAs.

Padding: pad columns get -30000 via the selector's extra all-ones row,
forcing h1=0 there, so their sigmoid output is exactly sigmoid(b4);
the host subtracts that known constant.
"""

import math
import os

import numpy as np
import ml_dtypes

BF16 = ml_dtypes.bfloat16

B, C, H, W = 4, 8, 384, 384
CH = C * H            # 3072 features per column
P = 128               # SBUF partitions
KT = CH // P          # 24 contraction tiles
CHANS = 18            # MLP width
NCORES = 8
CHUNK_KS = (4, 5, 6, 4, 3, 2)   # k-tiles per stream chunk: small first
#                                   chunk (early compute start) and small
#                                   last chunk (short trailing chain)
NCHUNK = len(CHUNK_KS)
ACT_IM = (2, 3)       # chunks whose im^2 runs on ACT (Square) instead of DVE

_prog_cache: dict = {}
LAST_RESULTS = None   # BassKernelResults of the most recent run (for test.py)


def _cst_layout(NWC, NS, NF):
    """Column offsets of the constant block [128, CW] (bf16)."""
    o = {}
    off = 0
    o["W1Q"] = off; off += KT * CHANS         # per-k W1q_k lhsT
    o["W1A"] = off; off += KT * CHANS         # per-k W1a_k lhsT
    o["W2BD"] = off; off += P                 # block-diag W2.T
    o["W3BD"] = off; off += P
    o["W4BD"] = off; off += 4
    o["REP"] = off; off += P                  # rows 0:18, eye at 32j offsets
    o["PADP"] = off; off += NWC               # rows 0:18: -30000 at pad cols
    o["B1"] = off; off += 1
    o["B2"] = off; off += 1
    o["B3"] = off; off += 1
    o["B4"] = off; off += 1
    o["CW"] = off
    return o


def _build_program(NWC: int, NL: int):
    """Build the SPMD Bass/Tile program for one core.

    NWC: number of wc (acquired) columns this core handles.
    NL:  wi slots per partition-quadrant (total wi slots = 4*NL).
    """
    import concourse.bass as bass
    import concourse.tile as tile
    from concourse import bacc, mybir

    f32 = mybir.dt.float32
    bf16 = mybir.dt.bfloat16
    NS = 4 * NL          # wi slots
    NCOL = NWC + NS      # feature columns per k-tile (a block then q block)
    NF = NL * NWC        # free columns of the pair block
    o = _cst_layout(NWC, NS, NF)
    CW = o["CW"]
    KOFF = [sum(CHUNK_KS[:c]) for c in range(NCHUNK + 1)]
    assert NF <= 512

    nc = bacc.Bacc("TRN2", debug=False)

    # ---- DRAM I/O (per-core; host packs exactly these layouts) ----
    # xks partition row: [chunk][re/im][k-in-chunk][col] -> contiguous 1D
    xks = nc.dram_tensor("xks", [P, 2 * KT * NCOL], bf16, kind="ExternalInput")
    cst = nc.dram_tensor("cst", [P, CW], bf16, kind="ExternalInput")
    hp = nc.dram_tensor("hp", [4, NL], f32, kind="ExternalOutput")

    AF = mybir.ActivationFunctionType
    ALU = mybir.AluOpType

    halves = (NL % 2 == 0 and NL >= 2)
    with tile.TileContext(nc) as tc:
        with (
            tc.tile_pool(name="consts", bufs=1) as consts,
            tc.tile_pool(name="xdata", bufs=NCHUNK) as xdata,
            tc.tile_pool(name="sq", bufs=2) as sqp,
            tc.tile_pool(name="feat", bufs=1) as featp,
            tc.tile_pool(name="mlp", bufs=1) as mlp,
            tc.tile_pool(name="ps1", bufs=1, space="PSUM") as ps1,
            tc.tile_pool(name="psa", bufs=1, space="PSUM") as psap,
            tc.tile_pool(name="psT", bufs=1, space="PSUM") as psT,
            tc.tile_pool(name="psH", bufs=1, space="PSUM") as psH,
            tc.tile_pool(name="ps2", bufs=1, space="PSUM") as ps2p,
            tc.tile_pool(name="ps3", bufs=1, space="PSUM") as ps3p,
            tc.tile_pool(name="ps4", bufs=1, space="PSUM") as ps4p,
        ):
            # ---- ACT sqrt-table prefetch before any data lands ----
            dum = mlp.tile([1, 1], f32, tag="dum")
            nc.vector.memset(dum, 1.0)
            dsq = mlp.tile([1, 1], f32, tag="dsq")
            nc.scalar.sqrt(dsq, dum)
            # L: per-slot lhsT for the Pq scatter matmul; row 32j+lw
            # gets pqT quadrant block j (32-aligned partition bases)
            L = mlp.tile([P, P], bf16, tag="L")
            nc.vector.memset(L, 0.0)
            # Sel[32j+lw, lw*NWC+c] = 1 selector, built on device while
            # the DMAs stream: iota+compare for one 32-row quadrant, then
            # aligned partition copies
            Sel = mlp.tile([P, NF], bf16, tag="Sel")
            sit = mlp.tile([32, NF], bf16, tag="sit")
            nc.gpsimd.iota(sit.rearrange("p (l c) -> p l c", l=NL),
                           pattern=[[1, NL], [0, NWC]], base=0,
                           channel_multiplier=-1,
                           allow_small_or_imprecise_dtypes=True)
            nc.vector.tensor_scalar(out=Sel[0:32, :], in0=sit, scalar1=0.0,
                                    scalar2=None, op0=ALU.is_equal)
            for j in range(1, 4):
                nc.vector.tensor_copy(Sel[32 * j:32 * (j + 1), :], Sel[0:32, :])

            # ---- input DMAs: all chunks on the sync ring in stream
            # order (chunk0 must land first; sharing rings only delays
            # it), constants on the scalar ring ----
            xc = []
            for ci in range(NCHUNK):
                cw = CHUNK_KS[ci] * NCOL
                t = xdata.tile([P, 2, cw], bf16, tag=f"x{ci}", name=f"x{ci}")
                a = 2 * KOFF[ci] * NCOL
                nc.sync.dma_start(
                    out=t, in_=xks[:, a:a + 2 * cw].rearrange(
                        "p (r n) -> p r n", r=2))
                xc.append(t)
            cst_s = consts.tile([P, CW], bf16, tag="cst")
            nc.scalar.dma_start(out=cst_s, in_=cst[:])

            # ---- features: |z| = sqrt(re^2 + im^2), then W1 matmuls.
            # DVE does all re^2 + most im^2 + all adds; ACT (Square shares
            # every table with Sqrt, so no table swaps) takes the middle
            # chunks' im^2 so the last chunk's chain stays on DVE (ACT
            # would otherwise trail). Separate q/a psums so Pq's pipeline
            # can start before the last a-matmuls retire. ----
            feat = featp.tile([P, KT * NCOL], bf16, tag="feat")
            psq = ps1.tile([CHANS, NS], f32, tag="psq")
            psa = psap.tile([CHANS, NWC], f32, tag="psa")
            sq_re = [sqp.tile([P, CHUNK_KS[ci] * NCOL], bf16, tag=f"sqre{ci}",
                              name=f"sqre{ci}") for ci in range(NCHUNK)]
            sq_im = [sqp.tile([P, CHUNK_KS[ci] * NCOL], bf16, tag=f"sqim{ci}",
                              name=f"sqim{ci}") for ci in range(NCHUNK)]
            m2 = [sqp.tile([P, CHUNK_KS[ci] * NCOL], bf16, tag=f"m2{ci}",
                           name=f"m2{ci}") for ci in range(NCHUNK)]

            def mm_a(k):
                nc.tensor.matmul(
                    out=psa,
                    lhsT=cst_s[:, o["W1A"] + k * CHANS:
                               o["W1A"] + (k + 1) * CHANS],
                    rhs=feat[:, k * NCOL:k * NCOL + NWC],
                    start=(k == 0), stop=(k == KT - 1))

            def mm_q(k):
                nc.tensor.matmul(
                    out=psq,
                    lhsT=cst_s[:, o["W1Q"] + k * CHANS:
                               o["W1Q"] + (k + 1) * CHANS],
                    rhs=feat[:, k * NCOL + NWC:(k + 1) * NCOL],
                    start=(k == 0), stop=(k == KT - 1))

            def emit_mms(ci):
                ks = range(KOFF[ci], KOFF[ci + 1])
                # last chunk: q matmuls first so the Pq tail starts early
                if ci == NCHUNK - 1:
                    for k in ks:
                        mm_q(k)
                    for k in ks:
                        mm_a(k)
                else:
                    for k in ks:
                        mm_a(k)
                    for k in ks:
                        mm_q(k)

            def dve_sq(ci, im):
                nc.vector.tensor_mul(sq_re[ci], xc[ci][:, 0], xc[ci][:, 0])
                if im:
                    nc.vector.tensor_mul(sq_im[ci], xc[ci][:, 1], xc[ci][:, 1])

            def dve_add(ci):
                nc.vector.tensor_add(m2[ci], sq_re[ci], sq_im[ci])

            def act_sqrt(ci):
                nc.scalar.sqrt(
                    feat[:, KOFF[ci] * NCOL:KOFF[ci + 1] * NCOL], m2[ci])

            actim = [ci for ci in ACT_IM if ci < NCHUNK]
            for ci in range(NCHUNK):
                on_dve = ci not in actim
                dve_sq(ci, on_dve)
                if on_dve:
                    dve_add(ci)
                    act_sqrt(ci)
                    # slot an ACT square between sqrts while data streams
                    if actim:
                        nc.scalar.square(sq_im[actim[0]], xc[actim[0]][:, 1])
                        actim.pop(0)
                else:
                    dve_add(ci)
                    act_sqrt(ci)
                emit_mms(ci)

            # sigmoid ACT-table prefetch: chained on the last sqrt's output
            # so it runs after all sqrts (exactly one table swap)
            dsg = mlp.tile([1, 1], f32, tag="dsg")
            nc.scalar.activation(out=dsg, in_=feat[0:1, KT * NCOL - 1:KT * NCOL],
                                 func=AF.Sigmoid)

            # biases as f32 (scalar operands must be float32); on gpsimd
            # so the wait doesn't block the DVE queue
            bias_f = mlp.tile([P, 4], f32, tag="biasf")
            nc.gpsimd.tensor_copy(bias_f, cst_s[:, o["B1"]:o["B1"] + 4])
            b1_s = bias_f[0:CHANS, 0:1]
            b2_s = bias_f[:, 1:2]
            b3_s = bias_f[:, 2:3]
            b4_s = bias_f[0:4, 3:4]

            # ---- extract Pq (+b1) and Pa from their psums ----
            pq_s = mlp.tile([CHANS, NS], bf16, tag="pq")
            nc.vector.tensor_scalar(out=pq_s, in0=psq,
                                    scalar1=b1_s, scalar2=None, op0=ALU.add)

            # ---- Pq quadrant scatter without DMA: 4 per-quadrant PE
            # transposes into one PSUM bank (base 0), then 32-aligned
            # copies into L ----
            # Pa extraction first (ready as soon as the a-psum stops);
            # the add also injects -30000 into pad columns
            pa_s = mlp.tile([CHANS, NWC], bf16, tag="pa")
            nc.vector.tensor_add(pa_s, psa,
                                 cst_s[0:CHANS, o["PADP"]:o["PADP"] + NWC])
            pqT = psT.tile([NL, 4 * CHANS], bf16, tag="pqT")
            eye = cst_s[0:CHANS, o["REP"]:o["REP"] + CHANS]
            for j in range(4):
                nc.tensor.matmul(
                    out=pqT[:, j * CHANS:(j + 1) * CHANS],
                    lhsT=pq_s[:, j * NL:(j + 1) * NL], rhs=eye,
                    is_transpose=True, start=(j == 0), stop=True,
                    skip_group_check=True)
            for j in range(4):
                nc.vector.tensor_copy(
                    L[32 * j:32 * j + NL, 32 * j:32 * j + CHANS],
                    pqT[:, j * CHANS:(j + 1) * CHANS])

            # ---- H = Pa (replicated, includes pad penalty; pa is ready
            # before L) + Pq (scattered). The Pq matmul is split by
            # column halves so the first relu can start while the second
            # half is still accumulating. ----
            Hps = psH.tile([P, NF], f32, tag="H")
            HNF = NF // 2 if (NL % 2 == 0 and NL >= 2) else NF
            for lw in range(NL):
                nc.tensor.matmul(out=Hps[:, lw * NWC:(lw + 1) * NWC],
                                 lhsT=cst_s[0:CHANS, o["REP"]:o["REP"] + P],
                                 rhs=pa_s, start=(lw == 0), stop=False,
                                 skip_group_check=True)
            for a in range(0, NF, HNF):
                nc.tensor.matmul(out=Hps[:, a:a + HNF], lhsT=L,
                                 rhs=Sel[:, a:a + HNF],
                                 start=False, stop=True,
                                 skip_group_check=True)

            # ---- pair MLP (two column-half pipelines when NL is even) ----
            h1 = mlp.tile([P, NF], bf16, tag="h1")
            h2 = mlp.tile([P, NF], bf16, tag="h2")
            h3 = mlp.tile([P, NF], bf16, tag="h3")
            sig = mlp.tile([4, NF], bf16, tag="sig")
            psum2 = ps2p.tile([P, NF], f32, tag="ps2")
            psum3 = ps3p.tile([P, NF], f32, tag="ps3")
            psum4 = ps4p.tile([4, NF], f32, tag="ps4")
            hp_s = mlp.tile([4, NL], f32, tag="hps")
            HNL = NL // 2 if halves else NL
            for hf in range(2 if halves else 1):
                sl = slice(hf * HNF, hf * HNF + HNF)
                nc.vector.tensor_scalar(out=h1[:, sl], in0=Hps[:, sl],
                                        scalar1=0.0, scalar2=None, op0=ALU.max)
                nc.tensor.matmul(out=psum2[:, sl],
                                 lhsT=cst_s[:, o["W2BD"]:o["W2BD"] + P],
                                 rhs=h1[:, sl], start=(hf == 0), stop=True,
                                 skip_group_check=True)
                # h2 relu on ACT (Relu lives in every table; ACT is idle
                # here and this decouples the two half-pipelines)
                nc.scalar.activation(out=h2[:, sl], in_=psum2[:, sl],
                                     func=AF.Relu, bias=b2_s, scale=1.0)
                nc.tensor.matmul(out=psum3[:, sl],
                                 lhsT=cst_s[:, o["W3BD"]:o["W3BD"] + P],
                                 rhs=h2[:, sl], start=(hf == 0), stop=True,
                                 skip_group_check=True)
                nc.vector.tensor_scalar(out=h3[:, sl], in0=psum3[:, sl],
                                        scalar1=b3_s, scalar2=0.0,
                                        op0=ALU.add, op1=ALU.max)
                nc.tensor.matmul(out=psum4[:, sl],
                                 lhsT=cst_s[:, o["W4BD"]:o["W4BD"] + 4],
                                 rhs=h3[:, sl], start=(hf == 0), stop=True,
                                 skip_group_check=True)
                nc.scalar.activation(out=sig[:, sl], in_=psum4[:, sl],
                                     func=AF.Sigmoid, bias=b4_s, scale=1.0)
                # heat[j, lw] = sum_c sig[j, lw*NWC + c]
                nc.vector.reduce_sum(
                    hp_s[:, hf * HNL:hf * HNL + HNL],
                    sig[:, sl].rearrange("p (l c) -> p l c", l=HNL),
                    axis=mybir.AxisListType.X)
            nc.sync.dma_start(out=hp[:], in_=hp_s)

    nc.finalize()
    return nc


def _run_sim(nc, in_maps):
    """CoreSim (CPU instruction simulator) path for local dev testing."""
    from concourse.bass_interp import MultiCoreSim
    from concourse.bass_utils import BassKernelResults

    sim = MultiCoreSim(nc, num_cores=len(in_maps))
    for core_id, core in sim.cores.items():
        for name, arr in in_maps[core_id].items():
            core.tensor(name)[:] = arr
    sim.simulate()
    results = [
        {"hp": np.array(sim.cores[i].tensor("hp"))} for i in range(len(in_maps))
    ]
    return BassKernelResults(results=results, instructions_and_trace=None,
                             profile_json=None, exec_time_ns=None)


def _mask_geometry(acquired_mask, acquiring_mask):
    """Replicates the reference's left/right/cmask/denom logic exactly."""
    am = np.asarray(acquired_mask, np.float32)
    qm = np.asarray(acquiring_mask, np.float32)
    mid = W // 2
    right = mid + np.argmax(am[:, mid:] < 1.0, axis=1)
    left = np.argmax(am[:, :mid][:, ::-1] < 1.0, axis=1) + 1
    cols = np.arange(W)
    cmask = (cols[None, :] >= left[:, None]) & (cols[None, :] < right[:, None])
    denom = (right - left).astype(np.float32)
    active = [np.nonzero(qm[b] > 0)[0] for b in range(B)]
    return left.astype(int), right.astype(int), cmask, denom, active


def kernel(acquired_kspace, acquiring_kspace, acquired_mask, acquiring_mask,
           W1, b1, W2, b2, W3, b3, W4, b4):
    global LAST_RESULTS
    from concourse.bass_utils import run_bass_kernel_spmd

    acquired_kspace = np.asarray(acquired_kspace, np.float32)
    acquiring_kspace = np.asarray(acquiring_kspace, np.float32)
    W1 = np.asarray(W1, np.float32)
    b1 = np.asarray(b1, np.float32)
    W2 = np.asarray(W2, np.float32)
    b2 = np.asarray(b2, np.float32)
    W3 = np.asarray(W3, np.float32)
    b3 = np.asarray(b3, np.float32)
    W4 = np.asarray(W4, np.float32)
    b4 = np.asarray(b4, np.float32)

    left, right, cmask, denom, active = _mask_geometry(acquired_mask, acquiring_mask)

    nmax = max(len(a) for a in active)
    out = np.zeros((B, H, W), np.float32)
    if nmax == 0:
        return out

    span = max(int((right - left).max()), 1)
    NL = max(1, math.ceil(nmax / 4))          # wi slots per quadrant
    NWC = max(1, math.ceil(span / 2))         # wc columns per core
    NS = 4 * NL
    NF = NL * NWC
    NCOL = NWC + NS
    assert NF <= 512, (NL, NWC)

    o = _cst_layout(NWC, NS, NF)

    # ---- shared constant block [128, CW] bf16 ----
    W1q, W1a = W1[:, :CH], W1[:, CH:]
    cstv = np.zeros((P, o["CW"]), np.float32)
    # separate per-k lhsT blocks: [p, k*18 + j] = W1x[j, k*128+p]
    w1q_t = W1q.T.reshape(KT, P, CHANS)       # [k, p, j]
    w1a_t = W1a.T.reshape(KT, P, CHANS)
    cstv[:, o["W1Q"]:o["W1Q"] + KT * CHANS] = (
        w1q_t.transpose(1, 0, 2).reshape(P, KT * CHANS))
    cstv[:, o["W1A"]:o["W1A"] + KT * CHANS] = (
        w1a_t.transpose(1, 0, 2).reshape(P, KT * CHANS))
    for j in range(4):
        sl = slice(32 * j, 32 * j + CHANS)
        cstv[sl, o["W2BD"] + 32 * j:o["W2BD"] + 32 * j + CHANS] = W2.T
        cstv[sl, o["W3BD"] + 32 * j:o["W3BD"] + 32 * j + CHANS] = W3.T
        cstv[sl, o["W4BD"] + j] = W4[0]
        cstv[sl, o["B2"]] = b2
        cstv[sl, o["B3"]] = b3
        cstv[:CHANS, o["REP"] + 32 * j:o["REP"] + 32 * j + CHANS] = np.eye(
            CHANS, dtype=np.float32)
    cstv[:CHANS, o["B1"]] = b1
    cstv[:4, o["B4"]] = float(b4[0])

    # ---- per-core packed inputs ----
    in_maps = []
    meta = []
    npad = np.zeros((B, 2), np.int64)
    for b in range(B):
        aw = active[b]
        awp = np.zeros(NS, np.int64)
        if len(aw):
            awp[:len(aw)] = aw
            awp[len(aw):] = aw[0]
        # acquiring features for active wi columns: [KT, P, NS, 2]
        Q = acquiring_kspace[b].reshape(CH, W, 2)[:, awp, :].reshape(
            KT, P, NS, 2)
        for s in range(2):
            w0 = int(left[b]) + s * NWC
            w1e = max(min(w0 + NWC, int(right[b])), w0)
            nv = w1e - w0
            npad[b, s] = NWC - nv
            A = np.zeros((KT, P, NWC, 2), np.float32)
            if nv > 0:
                A[:, :, :nv, :] = acquired_kspace[b].reshape(CH, W, 2)[
                    :, w0:w1e, :].reshape(KT, P, nv, 2)
            F = np.concatenate([A, Q], axis=2)      # [KT, P, NCOL, 2]
            # -> per chunk [P, r, k-in-chunk, col], chunk-major flattened
            parts = []
            koff = 0
            for kc in CHUNK_KS:
                Xc = F[koff:koff + kc].transpose(1, 3, 0, 2)  # [P, 2, kc, NCOL]
                parts.append(Xc.reshape(P, 2 * kc * NCOL))
                koff += kc
            xarr = np.ascontiguousarray(
                np.concatenate(parts, axis=1)).astype(BF16)
            cstc = cstv.copy()
            if nv < NWC:
                cstc[0:CHANS, o["PADP"] + nv:o["PADP"] + NWC] = -30000.0
            in_maps.append(dict(xks=xarr, cst=cstc.astype(BF16)))
            meta.append((b, s))

    key = (NWC, NL)
    if key not in _prog_cache:
        _prog_cache[key] = _build_program(NWC, NL)
    nc = _prog_cache[key]

    trace = bool(int(os.environ.get("CABSK_TRACE", "0")))
    tmpdir = os.environ.get("CABSK_TMPDIR") or None
    if tmpdir:
        import tempfile
        tmpdir = tempfile.mkdtemp(dir=tmpdir)
    if os.environ.get("CABSK_SIM", "0") == "1":
        res = _run_sim(nc, in_maps)
    else:
        res = run_bass_kernel_spmd(nc, in_maps, core_ids=list(range(NCORES)),
                                   trace=trace, tmpdir=tmpdir)
    LAST_RESULTS = res

    sig_b4 = 1.0 / (1.0 + math.exp(-float(b4[0])))  # pad columns' output
    heat = np.zeros((B, W), np.float32)
    for ci, (b, s) in enumerate(meta):
        hpv = np.asarray(res.results[ci]["hp"], np.float32)   # [4, NL]
        aw = active[b]
        corr = float(npad[b, s]) * sig_b4
        for t in range(len(aw)):
            heat[b, aw[t]] += hpv[t // NL, t % NL] - corr
    heat /= np.maximum(denom, 1.0)[:, None]
    out[:] = heat[:, None, :]
    return out
